# revision 15
# baseline (speedup 1.0000x reference)
"""AGNN (2-layer attention GNN) distributed Bass kernel for 8 TRN2 NeuronCores.

Pipeline (per core, dst-sharded):
  h0 = relu(x @ W1 + b1)                      (node shard, x streamed bf16)
  conv1: h1[i] = sum_e softmax_e(cos(xn_s, xn_d)) * h0[src]
  conv2: same with beta2
  out = h2 @ W2 + b2

Graph prep on host: nodes degree-sorted, dealt round-robin to 8 cores;
per-dst padded slot tables (cross-core-uniform K_t per tile) drive an
indirect-DMA gather from an all-gathered bf16 payload table
[xn(32) | inv_norm(1) | pad(1)] per node.  Segment softmax uses the
self-loop bound (max logit == |beta|), so no segment-max pass is needed.
"""

import os
import sys
import types

sys.path.insert(0, "/opt/trn_rl_repo")

import numpy as np
import ml_dtypes

import concourse.bass as bass
import concourse.bacc as bacc
import concourse.tile as tile
import concourse.mybir as mybir
from concourse.bass import IndirectOffsetOnAxis
from concourse.bass_utils import run_bass_kernel_spmd
from concourse.masks import make_identity

F32 = mybir.dt.float32
BF16 = mybir.dt.bfloat16
I32 = mybir.dt.int32
AF = mybir.ActivationFunctionType
ALU = mybir.AluOpType
AX = mybir.AxisListType

NC = 8
FIN = 500
FPAD = 512
HID = 32
NCLS = 40
PW = 34          # payload row width (bf16): 32 xn + 1 invr + 1 pad
EPS_NORM = 1e-12
EPS_SM = 1e-16


# ---------------------------------------------------------------- host prep

def _prep_graph(n, edge_index):
    """Degree-sort nodes, deal round-robin to cores, build padded slot tables."""
    src = np.asarray(edge_index[0], dtype=np.int64)
    dst = np.asarray(edge_index[1], dtype=np.int64)
    keep = src != dst                           # self-loops handled on-chip
    src, dst = src[keep], dst[keep]
    e = src.shape[0]

    mself = np.bincount(np.asarray(edge_index[1])[
        np.asarray(edge_index[0]) == np.asarray(edge_index[1])], minlength=n)
    deg = np.bincount(dst, minlength=n)
    order = np.argsort(-deg, kind="stable")          # global rank -> node id
    rank = np.empty(n, dtype=np.int64)
    rank[order] = np.arange(n)

    nsh = -(-n // NC)                                 # nodes per core shard
    tiles = -(-nsh // 128)
    nloc = tiles * 128

    deg_sorted = deg[order]
    ks = []
    for t in range(tiles):
        w0 = t * 128 * NC
        k = int(deg_sorted[w0]) if w0 < n else 1
        ks.append(max(k, 1))
    ks = np.array(ks, dtype=np.int64)
    tile_base = np.concatenate([[0], np.cumsum(128 * ks)])
    s_slots = int(tile_base[-1])

    # slot index for each edge
    r_dst = rank[dst]
    eorder = np.argsort(r_dst, kind="stable")
    rs = r_dst[eorder]
    ssrc = src[eorder]
    cum = np.concatenate([[0], np.cumsum(deg_sorted)])
    k_in_dst = np.arange(e, dtype=np.int64) - cum[rs]

    core = rs % NC
    local = rs // NC
    t_of = local // 128
    p_of = local % 128
    j = tile_base[t_of] + p_of * ks[t_of] + k_in_dst

    # table position (payload row) of each node: owner core block + local rank
    tpos = (rank % NC) * nloc + rank // NC

    offs = np.zeros((NC, s_slots), dtype=np.int32)
    msk = np.zeros((NC, s_slots), dtype=np.float32)
    offs[core, j] = tpos[ssrc].astype(np.int32)
    msk[core, j] = 1.0

    msl = np.zeros((NC, 128, tiles), dtype=np.float32)
    cr = np.arange(n)
    msl[rank % NC, (rank // NC) % 128, (rank // NC) // 128] = mself[cr]
    return dict(order=order, rank=rank, nsh=nsh, tiles=tiles, nloc=nloc,
                ks=[int(k) for k in ks], s_slots=s_slots, offs=offs, msk=msk,
                msl=msl)


# ---------------------------------------------------------------- device build

def _build(tiles, nloc, ks, s_slots, stage=99):
    nc = bacc.Bacc("TRN2", target_bir_lowering=False, debug=False,
                   num_devices=NC)

    xT = nc.dram_tensor("xT", [FPAD, nloc], BF16, kind="ExternalInput")
    W1 = nc.dram_tensor("W1", [FPAD, HID], BF16, kind="ExternalInput")
    b1b = nc.dram_tensor("b1b", [128, HID], F32, kind="ExternalInput")
    W2 = nc.dram_tensor("W2", [HID, NCLS], BF16, kind="ExternalInput")
    b2b = nc.dram_tensor("b2b", [128, NCLS], F32, kind="ExternalInput")
    bsc = nc.dram_tensor("bsc", [128, 4], F32, kind="ExternalInput")
    offs = nc.dram_tensor("offs", [s_slots], I32, kind="ExternalInput")
    msl = nc.dram_tensor("msl", [128, tiles], F32, kind="ExternalInput")
    msk = nc.dram_tensor("msk", [s_slots], F32, kind="ExternalInput")
    out = nc.dram_tensor("out", [nloc, NCLS], F32, kind="ExternalOutput")

    kmax = max(ks)
    tile_base = [0]
    for k in ks:
        tile_base.append(tile_base[-1] + 128 * k)

    with tile.TileContext(nc) as tc:
        with tc.tile_pool(name="const", bufs=1) as cpool, \
             tc.tile_pool(name="persist", bufs=1) as ppool, \
             tc.tile_pool(name="xio", bufs=3) as xpool, \
             tc.tile_pool(name="gio", bufs=3) as gpool, \
             tc.tile_pool(name="work", bufs=3) as wpool, \
             tc.tile_pool(name="small", bufs=4) as spool, \
             tc.tile_pool(name="ps", bufs=2, space="PSUM") as pspool, \
             tc.tile_pool(name="dram", bufs=1, space="DRAM") as dpool:

            # ---- constants to SBUF
            w1_sb = cpool.tile([128, 4, HID], BF16)
            nc.sync.dma_start(w1_sb[:], W1.ap().rearrange("(c p) h -> p c h", p=128))
            b1_sb = cpool.tile([128, HID], F32)
            nc.sync.dma_start(b1_sb[:], b1b.ap())
            w2_sb = cpool.tile([HID, NCLS], BF16)
            nc.sync.dma_start(w2_sb[:], W2.ap())
            b2_sb = cpool.tile([128, NCLS], F32)
            nc.sync.dma_start(b2_sb[:], b2b.ap())
            bsc_sb = cpool.tile([128, 4], F32)
            nc.sync.dma_start(bsc_sb[:], bsc.ap())
            msl_sb = cpool.tile([128, tiles], F32)
            nc.sync.dma_start(msl_sb[:], msl.ap())
            ident = cpool.tile([128, 128], F32)
            make_identity(nc, ident[:])
            c_eps = cpool.tile([128, 1], F32)
            nc.gpsimd.memset(c_eps[:], EPS_NORM)
            c_neg1 = cpool.tile([128, 1], F32)
            nc.gpsimd.memset(c_neg1[:], -1.0)

            # ---- persistent node state
            h_sb = ppool.tile([128, tiles, HID], F32)     # current features
            h_pr = ppool.tile([128, tiles, HID], F32)     # prev-layer features
            dt0 = ppool.tile([128, tiles, HID], BF16)     # xn for conv1 dst
            dt1 = ppool.tile([128, tiles, HID], BF16)     # xn for conv2 dst

            pay1 = dpool.tile([nloc, PW], BF16)
            pay2 = dpool.tile([nloc, PW], BF16)
            tab1 = dpool.tile([NC * nloc, PW], BF16)
            tab2 = dpool.tile([NC * nloc, PW], BF16)

            xT_re = xT.ap().rearrange("(c p) n -> p c n", p=128)

            def normalize_payload(h_ap, dt_ap, pay_dram, t, sub=99):
                """xn = h/|h|; payload row [xn | invr | junk] -> pay_dram."""
                hsq = spool.tile([128, HID], F32, tag="hsq")
                ss = spool.tile([128, 1], F32, tag="ss")
                nc.vector.tensor_tensor(hsq[:], h_ap, h_ap, ALU.mult)
                nc.vector.tensor_reduce(ss[:], hsq[:], AX.X, ALU.add)
                if sub <= 1:
                    nc.vector.tensor_copy(dt_ap, h_ap)
                    return
                lss = spool.tile([128, 1], F32, tag="lss")
                nc.scalar.activation(lss[:], ss[:], AF.Ln, bias=c_eps[:])
                invr = spool.tile([128, 1], F32, tag="invr")
                nc.scalar.activation(invr[:], lss[:], AF.Exp, scale=-0.5)
                rr = spool.tile([128, 1], F32, tag="rr")
                nc.scalar.activation(rr[:], lss[:], AF.Exp, scale=0.5)
                if sub <= 2:
                    nc.vector.tensor_copy(dt_ap, h_ap)
                    return
                xn = spool.tile([128, HID], F32, tag="xn")
                nc.vector.tensor_scalar_mul(xn[:], h_ap, invr[:])
                payt = spool.tile([128, PW], BF16, tag="payt")
                nc.vector.tensor_copy(payt[:, 0:HID], xn[:])
                nc.vector.tensor_copy(payt[:, HID:HID + 2],
                                      rr[:].to_broadcast([128, 2]))
                nc.vector.tensor_copy(dt_ap, xn[:])
                if sub <= 3:
                    return
                nc.gpsimd.dma_start(pay_dram[t * 128:(t + 1) * 128, :],
                                    payt[:])

            def dump_h(t, ap32, cast=False):
                ot = spool.tile([128, NCLS], F32, tag="ot_dbg")
                nc.gpsimd.memset(ot[:], 0.0)
                nc.vector.tensor_copy(ot[:, 0:HID], ap32)
                nc.sync.dma_start(out.ap()[t * 128:(t + 1) * 128, :], ot[:])

            # ================= layer 1 + payload for conv1
            for t in range(tiles):
                xt = xpool.tile([128, 4, 128], BF16, tag="xt")
                nc.sync.dma_start(xt[:], xT_re[:, :, t * 128:(t + 1) * 128])
                ps = pspool.tile([128, HID], F32, tag="ps1")
                for c in range(4):
                    nc.tensor.matmul(ps[:], lhsT=xt[:, c, :], rhs=w1_sb[:, c, :],
                                     start=(c == 0), stop=(c == 3))
                h_t = h_sb[:, t, :]
                nc.vector.tensor_tensor(h_t, ps[:], b1_sb[:], ALU.add)
                nc.vector.tensor_relu(h_t, h_t)
                if stage == 1:
                    dump_h(t, h_t)
                    continue
                if 11 <= stage <= 15:
                    normalize_payload(h_t, dt0[:, t, :], pay1, t, sub=stage - 11)
                    nc.vector.tensor_copy(dt1[:, t, :], h_t)
                    dump_h(t, h_t)
                    continue
                normalize_payload(h_t, dt0[:, t, :], pay1, t)

            if stage == 1:
                nc.compile_marker = True
            if stage >= 2 and not (11 <= stage <= 15):
                nc.gpsimd.collective_compute(
                "AllGather", ALU.bypass,
                    replica_groups=[list(range(NC))],
                    ins=[pay1.opt()], outs=[tab1.opt()])
            if stage == 2:
                for t in range(tiles):
                    dump_h(t, h_sb[:, t, :])

            # ================= conv layer
            def conv(table, dts, scale_arg, bias_arg, wself_arg, h_prev, epilogue):
                for t in range(tiles):
                    k = ks[t]
                    base = tile_base[t]
                    off_t = gpool.tile([128, kmax], I32, tag="off")
                    nc.sync.dma_start(
                        off_t[:, 0:k],
                        offs.ap()[base:base + 128 * k]
                            .rearrange("(p k) -> p k", k=k))
                    msk_t = gpool.tile([128, kmax], F32, tag="msk")
                    nc.sync.dma_start(
                        msk_t[:, 0:k],
                        msk.ap()[base:base + 128 * k]
                            .rearrange("(p k) -> p k", k=k))
                    g = gpool.tile([128, kmax * PW], BF16, tag="g")
                    for kk in range(k):
                        nc.gpsimd.indirect_dma_start(
                            out=g[:, kk * PW:(kk + 1) * PW], out_offset=None,
                            in_=table[:],
                            in_offset=IndirectOffsetOnAxis(
                                ap=off_t[:, kk:kk + 1], axis=0))
                    g3 = g[:, 0:k * PW].rearrange("p (k d) -> p k d", d=PW)

                    if stage == 3:
                        agg0 = spool.tile([128, HID], F32, tag="agg")
                        nc.vector.tensor_reduce(
                            agg0[:],
                            g[:, 0:k * PW].rearrange("p (k d) -> p d k", d=PW)[:, 0:HID, :],
                            AX.X, ALU.add)
                        dump_h(t, agg0[:])
                        continue

                    pl = wpool.tile([128, kmax * HID], BF16, tag="pl")
                    pl3 = pl[:, 0:k * HID].rearrange("p (k f) -> p k f",
                                                          f=HID)
                    dbc = dts[:, t, :].unsqueeze(1).to_broadcast([128, k, HID])
                    nc.vector.tensor_tensor(pl3, g3[:, :, 0:HID], dbc, ALU.mult)

                    ll = wpool.tile([128, kmax], F32, tag="ll")
                    nc.vector.tensor_reduce(ll[:, 0:k], pl3, AX.X, ALU.add)

                    w = wpool.tile([128, kmax], F32, tag="w")
                    nc.scalar.activation(w[:, 0:k], ll[:, 0:k], AF.Exp,
                                         bias=bias_arg, scale=scale_arg)
                    wm = wpool.tile([128, kmax], F32, tag="wm")
                    nc.vector.tensor_tensor(wm[:, 0:k], w[:, 0:k], msk_t[:, 0:k],
                                            ALU.mult)
                    z = spool.tile([128, 1], F32, tag="z")
                    nc.vector.tensor_reduce(z[:], wm[:, 0:k], AX.X, ALU.add)
                    nc.vector.tensor_scalar_add(z[:], z[:], EPS_SM)
                    # self-loop: z += wself * multiplicity
                    zs = spool.tile([128, 1], F32, tag="zs")
                    nc.vector.tensor_tensor(zs[:], msl_sb[:, t:t + 1],
                                            wself_arg, ALU.mult)
                    nc.vector.tensor_tensor(z[:], z[:], zs[:], ALU.add)
                    rz = spool.tile([128, 1], F32, tag="rz")
                    nc.vector.reciprocal(rz[:], z[:])

                    invc = wpool.tile([128, kmax], F32, tag="invc")
                    nc.vector.tensor_copy(invc[:, 0:k], g3[:, :, HID:HID + 1]
                                          .rearrange("p k d -> p (k d)"))
                    wp = wpool.tile([128, kmax], BF16, tag="wp")
                    nc.vector.scalar_tensor_tensor(
                        out=wp[:, 0:k], in0=wm[:, 0:k], scalar=1.0,
                        in1=invc[:, 0:k], op0=ALU.mult, op1=ALU.mult)

                    pa = wpool.tile([128, kmax * HID], BF16, tag="pa")
                    pa3 = pa[:, 0:k * HID].rearrange("p (k f) -> p k f",
                                                          f=HID)
                    wbc = wp[:, 0:k].unsqueeze(2).to_broadcast([128, k, HID])
                    nc.vector.tensor_tensor(pa3, g3[:, :, 0:HID], wbc, ALU.mult)

                    agg = spool.tile([128, HID], F32, tag="agg")
                    nc.vector.tensor_reduce(
                        agg[:],
                        pa[:, 0:k * HID].rearrange("p (k f) -> p f k", f=HID),
                        AX.X, ALU.add)
                    # self-loop: agg += wself * h_prev[tile]
                    hw = spool.tile([128, HID], F32, tag="hwself")
                    nc.vector.tensor_scalar_mul(hw[:], h_prev[:, t, :], zs[:])
                    nc.vector.tensor_add(agg[:], agg[:], hw[:])
                    h_new = spool.tile([128, HID], F32, tag="hnew")
                    nc.vector.tensor_scalar_mul(h_new[:], agg[:], rz[:])
                    epilogue(t, h_new)

            def ep_conv1(t, h_new):
                nc.vector.tensor_copy(h_pr[:, t, :], h_new[:])
                if stage == 4:
                    dump_h(t, h_new[:])
                    return
                normalize_payload(h_pr[:, t, :], dt1[:, t, :], pay2, t)

            if stage >= 3 and not (11 <= stage <= 15):
                conv(tab1, dt0, 1.0, c_neg1[:], bsc_sb[:, 2:3], h_sb, ep_conv1)

            if stage >= 5:
                nc.gpsimd.collective_compute(
                "AllGather", ALU.bypass,
                    replica_groups=[list(range(NC))],
                    ins=[pay2.opt()], outs=[tab2.opt()])

            def ep_conv2(t, h_new):
                tp = pspool.tile([HID, 128], F32, tag="tp")
                nc.tensor.transpose(out=tp[:], in_=h_new[:], identity=ident[:])
                h2t = spool.tile([HID, 128], BF16, tag="h2t")
                nc.vector.tensor_copy(h2t[:], tp[:])
                ps2 = pspool.tile([128, NCLS], F32, tag="ps2")
                nc.tensor.matmul(ps2[:], lhsT=h2t[:], rhs=w2_sb[:],
                                 start=True, stop=True)
                ot = spool.tile([128, NCLS], F32, tag="ot")
                nc.vector.tensor_tensor(ot[:], ps2[:], b2_sb[:], ALU.add)
                nc.sync.dma_start(out.ap()[t * 128:(t + 1) * 128, :], ot[:])

            if stage >= 5:
                conv(tab2, dt1, bsc_sb[:, 0:1], bsc_sb[:, 1:2],
                     bsc_sb[:, 3:4], h_pr, ep_conv2)

    nc.compile()
    return nc


# ---------------------------------------------------------------- entry point

def kernel(x, W1, b1, W2, b2, beta2, edge_index):
    x = np.asarray(x, dtype=np.float32)
    W1 = np.asarray(W1, dtype=np.float32)
    b1 = np.asarray(b1, dtype=np.float32)
    W2 = np.asarray(W2, dtype=np.float32)
    b2 = np.asarray(b2, dtype=np.float32)
    beta2 = float(np.asarray(beta2))
    edge_index = np.asarray(edge_index)

    n = x.shape[0]
    g = _prep_graph(n, edge_index)
    tiles, nloc, nsh = g["tiles"], g["nloc"], g["nsh"]

    nc = _build(tiles, nloc, g["ks"], g["s_slots"],
                stage=int(os.environ.get("AGNN_STAGE", "99")))

    w1p = np.zeros((FPAD, HID), dtype=ml_dtypes.bfloat16)
    w1p[:FIN] = W1.astype(ml_dtypes.bfloat16)
    b1b = np.broadcast_to(b1, (128, HID)).copy().astype(np.float32)
    w2 = W2.astype(ml_dtypes.bfloat16)
    b2b = np.broadcast_to(b2, (128, NCLS)).copy().astype(np.float32)
    bscv = np.zeros((128, 4), dtype=np.float32)
    bscv[:, 0] = beta2
    bscv[:, 1] = -abs(beta2)
    bscv[:, 2] = 1.0                            # wself conv1 (beta=1)
    bscv[:, 3] = np.exp(beta2 - abs(beta2))     # wself conv2

    order = g["order"]
    xbf = x.astype(ml_dtypes.bfloat16)
    in_maps = []
    for c in range(NC):
        ids = order[c::NC]
        xs = np.zeros((FPAD, nloc), dtype=ml_dtypes.bfloat16)
        xs[:FIN, :len(ids)] = xbf[ids].T
        in_maps.append({
            "xT": xs, "W1": w1p, "b1b": b1b, "W2": w2, "b2b": b2b,
            "bsc": bscv,
            "offs": g["offs"][c], "msk": g["msk"][c], "msl": g["msl"][c],
        })

    trace = os.environ.get("AGNN_TRACE", "") == "1"
    kwargs = {}
    if trace:
        _enable_ntff_hook()
        kwargs = dict(trace=True,
                      tmpdir=os.environ.get("AGNN_TRACE_DIR", "/tmp/agnn_trace"))
        os.makedirs(kwargs["tmpdir"], exist_ok=True)
    res = run_bass_kernel_spmd(nc, in_maps, core_ids=list(range(NC)), **kwargs)
    if trace:
        print("AGNN exec_time_ns:", res.exec_time_ns)
        kernel._last_exec_time_ns = res.exec_time_ns

    out_full = np.empty((n, NCLS), dtype=np.float32)
    for c in range(NC):
        ids = order[c::NC]
        out_full[ids] = res.results[c]["out"][:len(ids)]
    return out_full


def _enable_ntff_hook():
    import antenv
    if "antenv.axon_hooks" not in sys.modules:
        mod = types.ModuleType("antenv.axon_hooks")
        _h = [None]
        mod.set_axon_ntff_profile_hook = lambda v: _h.__setitem__(0, v)
        mod.get_axon_ntff_profile_hook = lambda: _h[0]
        sys.modules["antenv.axon_hooks"] = mod
        antenv.axon_hooks = mod
    import concourse.bass_utils as bu
    bu.upload_artifacts = lambda d: d
    from trn_agent_boot.trn_boot import _ntff_profile_via_ctypes
    sys.modules["antenv.axon_hooks"].set_axon_ntff_profile_hook(
        _ntff_profile_via_ctypes("/opt/axon/libaxon_pjrt.so"))


# revision 16
# speedup vs baseline: 1.1779x; 1.1779x over previous
"""AGNN (2-layer attention GNN) distributed Bass kernel for 8 TRN2 NeuronCores.

Pipeline (per core, dst-sharded):
  h0 = relu(x @ W1 + b1)                      (node shard, x streamed bf16)
  conv1: h1[i] = sum_e softmax_e(cos(xn_s, xn_d)) * h0[src]
  conv2: same with beta2
  out = h2 @ W2 + b2

Graph prep on host: nodes degree-sorted, dealt round-robin to 8 cores;
per-dst padded slot tables (cross-core-uniform K_t per tile) drive an
indirect-DMA gather from an all-gathered bf16 payload table
[xn(32) | inv_norm(1) | pad(1)] per node.  Segment softmax uses the
self-loop bound (max logit == |beta|), so no segment-max pass is needed.
"""

import os
import sys
import types

sys.path.insert(0, "/opt/trn_rl_repo")

import numpy as np
import ml_dtypes

import concourse.bass as bass
import concourse.bacc as bacc
import concourse.tile as tile
import concourse.mybir as mybir
from concourse.bass import IndirectOffsetOnAxis
from concourse.bass_utils import run_bass_kernel_spmd
from concourse.masks import make_identity

F32 = mybir.dt.float32
BF16 = mybir.dt.bfloat16
I32 = mybir.dt.int32
AF = mybir.ActivationFunctionType
ALU = mybir.AluOpType
AX = mybir.AxisListType

NC = 8
FIN = 500
FPAD = 512
HID = 32
NCLS = 40
PW = 34          # payload row width (bf16): 32 xn + 1 invr + 1 pad
EPS_NORM = 1e-12
EPS_SM = 1e-16


# ---------------------------------------------------------------- host prep

def _prep_graph(n, edge_index):
    """Degree-sort nodes, deal round-robin to cores, build padded slot tables."""
    src = np.asarray(edge_index[0], dtype=np.int64)
    dst = np.asarray(edge_index[1], dtype=np.int64)
    keep = src != dst                           # self-loops handled on-chip
    src, dst = src[keep], dst[keep]
    e = src.shape[0]

    mself = np.bincount(np.asarray(edge_index[1])[
        np.asarray(edge_index[0]) == np.asarray(edge_index[1])], minlength=n)
    deg = np.bincount(dst, minlength=n)
    order = np.argsort(-deg, kind="stable")          # global rank -> node id
    rank = np.empty(n, dtype=np.int64)
    rank[order] = np.arange(n)

    nsh = -(-n // NC)                                 # nodes per core shard
    tiles = -(-nsh // 128)
    nloc = tiles * 128

    deg_sorted = deg[order]
    ks = []
    for t in range(tiles):
        w0 = t * 128 * NC
        k = int(deg_sorted[w0]) if w0 < n else 1
        ks.append(max(k, 1))
    ks = np.array(ks, dtype=np.int64)
    tile_base = np.concatenate([[0], np.cumsum(128 * ks)])
    s_slots = int(tile_base[-1])

    # slot index for each edge
    r_dst = rank[dst]
    eorder = np.argsort(r_dst, kind="stable")
    rs = r_dst[eorder]
    ssrc = src[eorder]
    cum = np.concatenate([[0], np.cumsum(deg_sorted)])
    k_in_dst = np.arange(e, dtype=np.int64) - cum[rs]

    core = rs % NC
    local = rs // NC
    t_of = local // 128
    p_of = local % 128
    j = tile_base[t_of] + p_of * ks[t_of] + k_in_dst

    # table position (payload row) of each node: owner core block + local rank
    tpos = (rank % NC) * nloc + rank // NC

    offs = np.zeros((NC, s_slots), dtype=np.int32)
    msk = np.zeros((NC, s_slots), dtype=np.float32)
    offs[core, j] = tpos[ssrc].astype(np.int32)
    msk[core, j] = 1.0

    msl = np.zeros((NC, 128, tiles), dtype=np.float32)
    cr = np.arange(n)
    msl[rank % NC, (rank // NC) % 128, (rank // NC) // 128] = mself[cr]
    return dict(order=order, rank=rank, nsh=nsh, tiles=tiles, nloc=nloc,
                ks=[int(k) for k in ks], s_slots=s_slots, offs=offs, msk=msk,
                msl=msl)


# ---------------------------------------------------------------- device build

def _build(tiles, nloc, ks, s_slots, stage=99):
    nc = bacc.Bacc("TRN2", target_bir_lowering=False, debug=False,
                   num_devices=NC)

    xT = nc.dram_tensor("xT", [FPAD, nloc], BF16, kind="ExternalInput")
    W1 = nc.dram_tensor("W1", [FPAD, HID], BF16, kind="ExternalInput")
    b1b = nc.dram_tensor("b1b", [128, HID], F32, kind="ExternalInput")
    W2 = nc.dram_tensor("W2", [HID, NCLS], BF16, kind="ExternalInput")
    b2b = nc.dram_tensor("b2b", [128, NCLS], F32, kind="ExternalInput")
    bsc = nc.dram_tensor("bsc", [128, 4], F32, kind="ExternalInput")
    offs = nc.dram_tensor("offs", [s_slots], I32, kind="ExternalInput")
    msl = nc.dram_tensor("msl", [128, tiles], F32, kind="ExternalInput")
    msk = nc.dram_tensor("msk", [s_slots], F32, kind="ExternalInput")
    out = nc.dram_tensor("out", [nloc, NCLS], F32, kind="ExternalOutput")

    kmax = max(ks)
    tile_base = [0]
    for k in ks:
        tile_base.append(tile_base[-1] + 128 * k)

    with tile.TileContext(nc) as tc:
        with tc.tile_pool(name="const", bufs=1) as cpool, \
             tc.tile_pool(name="persist", bufs=1) as ppool, \
             tc.tile_pool(name="xio", bufs=3) as xpool, \
             tc.tile_pool(name="gio", bufs=3) as gpool, \
             tc.tile_pool(name="work", bufs=3) as wpool, \
             tc.tile_pool(name="small", bufs=4) as spool, \
             tc.tile_pool(name="ps", bufs=2, space="PSUM") as pspool, \
             tc.tile_pool(name="dram", bufs=1, space="DRAM") as dpool:

            # ---- constants to SBUF
            w1_sb = cpool.tile([128, 4, HID], BF16)
            nc.sync.dma_start(w1_sb[:], W1.ap().rearrange("(c p) h -> p c h", p=128))
            b1_sb = cpool.tile([128, HID], F32)
            nc.sync.dma_start(b1_sb[:], b1b.ap())
            w2_sb = cpool.tile([HID, NCLS], BF16)
            nc.sync.dma_start(w2_sb[:], W2.ap())
            b2_sb = cpool.tile([128, NCLS], F32)
            nc.sync.dma_start(b2_sb[:], b2b.ap())
            bsc_sb = cpool.tile([128, 4], F32)
            nc.sync.dma_start(bsc_sb[:], bsc.ap())
            msl_sb = cpool.tile([128, tiles], F32)
            nc.sync.dma_start(msl_sb[:], msl.ap())
            ident = cpool.tile([128, 128], F32)
            make_identity(nc, ident[:])
            c_eps = cpool.tile([128, 1], F32)
            nc.gpsimd.memset(c_eps[:], EPS_NORM)
            c_neg1 = cpool.tile([128, 1], F32)
            nc.gpsimd.memset(c_neg1[:], -1.0)

            # ---- persistent node state
            h_sb = ppool.tile([128, tiles, HID], F32)     # current features
            h_pr = ppool.tile([128, tiles, HID], F32)     # prev-layer features
            dt0 = ppool.tile([128, tiles, HID], BF16)     # xn for conv1 dst
            dt1 = ppool.tile([128, tiles, HID], BF16)     # xn for conv2 dst

            pay1 = dpool.tile([nloc, PW], BF16)
            pay2 = dpool.tile([nloc, PW], BF16)
            tab1 = dpool.tile([NC * nloc, PW], BF16)
            tab2 = dpool.tile([NC * nloc, PW], BF16)

            xT_re = xT.ap().rearrange("(c p) n -> p c n", p=128)

            def normalize_payload(h_ap, dt_ap, pay_dram, t, sub=99):
                """xn = h/|h|; payload row [xn | invr | junk] -> pay_dram."""
                hsq = spool.tile([128, HID], F32, tag="hsq")
                ss = spool.tile([128, 1], F32, tag="ss")
                nc.vector.tensor_tensor(hsq[:], h_ap, h_ap, ALU.mult)
                nc.vector.tensor_reduce(ss[:], hsq[:], AX.X, ALU.add)
                if sub <= 1:
                    nc.vector.tensor_copy(dt_ap, h_ap)
                    return
                lss = spool.tile([128, 1], F32, tag="lss")
                nc.scalar.activation(lss[:], ss[:], AF.Ln, bias=c_eps[:])
                invr = spool.tile([128, 1], F32, tag="invr")
                nc.scalar.activation(invr[:], lss[:], AF.Exp, scale=-0.5)
                rr = spool.tile([128, 1], F32, tag="rr")
                nc.scalar.activation(rr[:], lss[:], AF.Exp, scale=0.5)
                if sub <= 2:
                    nc.vector.tensor_copy(dt_ap, h_ap)
                    return
                xn = spool.tile([128, HID], F32, tag="xn")
                nc.vector.tensor_scalar_mul(xn[:], h_ap, invr[:])
                payt = spool.tile([128, PW], BF16, tag="payt")
                nc.vector.tensor_copy(payt[:, 0:HID], xn[:])
                nc.vector.tensor_copy(payt[:, HID:HID + 2],
                                      rr[:].to_broadcast([128, 2]))
                nc.vector.tensor_copy(dt_ap, xn[:])
                if sub <= 3:
                    return
                nc.gpsimd.dma_start(pay_dram[t * 128:(t + 1) * 128, :],
                                    payt[:])

            def dump_h(t, ap32, cast=False):
                ot = spool.tile([128, NCLS], F32, tag="ot_dbg")
                nc.gpsimd.memset(ot[:], 0.0)
                nc.vector.tensor_copy(ot[:, 0:HID], ap32)
                nc.sync.dma_start(out.ap()[t * 128:(t + 1) * 128, :], ot[:])

            # ================= layer 1 + payload for conv1
            for t in range(tiles):
                xt = xpool.tile([128, 4, 128], BF16, tag="xt")
                nc.sync.dma_start(xt[:], xT_re[:, :, t * 128:(t + 1) * 128])
                ps = pspool.tile([128, HID], F32, tag="ps1")
                for c in range(4):
                    nc.tensor.matmul(ps[:], lhsT=xt[:, c, :], rhs=w1_sb[:, c, :],
                                     start=(c == 0), stop=(c == 3))
                h_t = h_sb[:, t, :]
                nc.vector.tensor_tensor(h_t, ps[:], b1_sb[:], ALU.add)
                nc.vector.tensor_relu(h_t, h_t)
                if stage == 1:
                    dump_h(t, h_t)
                    continue
                if 11 <= stage <= 15:
                    normalize_payload(h_t, dt0[:, t, :], pay1, t, sub=stage - 11)
                    nc.vector.tensor_copy(dt1[:, t, :], h_t)
                    dump_h(t, h_t)
                    continue
                normalize_payload(h_t, dt0[:, t, :], pay1, t)

            if stage == 1:
                nc.compile_marker = True
            if stage >= 2 and not (11 <= stage <= 15):
                nc.gpsimd.collective_compute(
                "AllGather", ALU.bypass,
                    replica_groups=[list(range(NC))],
                    ins=[pay1.opt()], outs=[tab1.opt()])
            if stage == 2:
                for t in range(tiles):
                    dump_h(t, h_sb[:, t, :])

            # ================= conv layer
            def conv(table, dts, scale_arg, bias_arg, wself_arg, h_prev, epilogue):
                for t in range(tiles):
                    k = ks[t]
                    base = tile_base[t]
                    off_t = gpool.tile([128, kmax], I32, tag="off")
                    nc.sync.dma_start(
                        off_t[:, 0:k],
                        offs.ap()[base:base + 128 * k]
                            .rearrange("(p k) -> p k", k=k))
                    msk_t = gpool.tile([128, kmax], F32, tag="msk")
                    nc.sync.dma_start(
                        msk_t[:, 0:k],
                        msk.ap()[base:base + 128 * k]
                            .rearrange("(p k) -> p k", k=k))
                    g = gpool.tile([128, kmax * PW], BF16, tag="g")
                    for kk in range(k):
                        nc.gpsimd.indirect_dma_start(
                            out=g[:, kk * PW:(kk + 1) * PW], out_offset=None,
                            in_=table[:],
                            in_offset=IndirectOffsetOnAxis(
                                ap=off_t[:, kk:kk + 1], axis=0))
                    g3 = g[:, 0:k * PW].rearrange("p (k d) -> p k d", d=PW)

                    if stage == 3:
                        agg0 = spool.tile([128, HID], F32, tag="agg")
                        nc.vector.tensor_reduce(
                            agg0[:],
                            g[:, 0:k * PW].rearrange("p (k d) -> p d k", d=PW)[:, 0:HID, :],
                            AX.X, ALU.add)
                        dump_h(t, agg0[:])
                        continue

                    pl = wpool.tile([128, kmax * HID], BF16, tag="pl")
                    pl3 = pl[:, 0:k * HID].rearrange("p (k f) -> p k f",
                                                          f=HID)
                    dbc = dts[:, t, :].unsqueeze(1).to_broadcast([128, k, HID])
                    nc.vector.tensor_tensor(pl3, g3[:, :, 0:HID], dbc, ALU.mult)

                    ll = wpool.tile([128, kmax], F32, tag="ll")
                    nc.vector.tensor_reduce(ll[:, 0:k], pl3, AX.X, ALU.add)

                    w = wpool.tile([128, kmax], F32, tag="w")
                    nc.scalar.activation(w[:, 0:k], ll[:, 0:k], AF.Exp,
                                         bias=bias_arg, scale=scale_arg)
                    wm = wpool.tile([128, kmax], F32, tag="wm")
                    nc.vector.tensor_tensor(wm[:, 0:k], w[:, 0:k], msk_t[:, 0:k],
                                            ALU.mult)
                    z = spool.tile([128, 1], F32, tag="z")
                    nc.vector.tensor_reduce(z[:], wm[:, 0:k], AX.X, ALU.add)
                    nc.vector.tensor_scalar_add(z[:], z[:], EPS_SM)
                    # self-loop: z += wself * multiplicity
                    zs = spool.tile([128, 1], F32, tag="zs")
                    nc.vector.tensor_tensor(zs[:], msl_sb[:, t:t + 1],
                                            wself_arg, ALU.mult)
                    nc.vector.tensor_tensor(z[:], z[:], zs[:], ALU.add)
                    rz = spool.tile([128, 1], F32, tag="rz")
                    nc.vector.reciprocal(rz[:], z[:])

                    invc = wpool.tile([128, kmax], F32, tag="invc")
                    nc.vector.tensor_copy(invc[:, 0:k], g3[:, :, HID:HID + 1]
                                          .rearrange("p k d -> p (k d)"))
                    wp = wpool.tile([128, kmax], BF16, tag="wp")
                    nc.vector.scalar_tensor_tensor(
                        out=wp[:, 0:k], in0=wm[:, 0:k], scalar=1.0,
                        in1=invc[:, 0:k], op0=ALU.mult, op1=ALU.mult)

                    pa = wpool.tile([128, kmax * HID], BF16, tag="pa")
                    pa3 = pa[:, 0:k * HID].rearrange("p (k f) -> p k f",
                                                          f=HID)
                    wbc = wp[:, 0:k].unsqueeze(2).to_broadcast([128, k, HID])
                    nc.vector.tensor_tensor(pa3, g3[:, :, 0:HID], wbc, ALU.mult)

                    agg = spool.tile([128, HID], F32, tag="agg")
                    nc.vector.tensor_reduce(
                        agg[:],
                        pa[:, 0:k * HID].rearrange("p (k f) -> p f k", f=HID),
                        AX.X, ALU.add)
                    # self-loop: agg += wself * h_prev[tile]
                    hw = spool.tile([128, HID], F32, tag="hwself")
                    nc.vector.tensor_scalar_mul(hw[:], h_prev[:, t, :], zs[:])
                    nc.vector.tensor_add(agg[:], agg[:], hw[:])
                    h_new = spool.tile([128, HID], F32, tag="hnew")
                    nc.vector.tensor_scalar_mul(h_new[:], agg[:], rz[:])
                    epilogue(t, h_new)

            def ep_conv1(t, h_new):
                nc.vector.tensor_copy(h_pr[:, t, :], h_new[:])
                if stage == 4:
                    dump_h(t, h_new[:])
                    return
                normalize_payload(h_pr[:, t, :], dt1[:, t, :], pay2, t)

            if stage >= 3 and not (11 <= stage <= 15):
                conv(tab1, dt0, 1.0, c_neg1[:], bsc_sb[:, 2:3], h_sb, ep_conv1)

            if stage >= 5:
                nc.gpsimd.collective_compute(
                "AllGather", ALU.bypass,
                    replica_groups=[list(range(NC))],
                    ins=[pay2.opt()], outs=[tab2.opt()])

            def ep_conv2(t, h_new):
                tp = pspool.tile([HID, 128], F32, tag="tp")
                nc.tensor.transpose(out=tp[:], in_=h_new[:], identity=ident[:])
                h2t = spool.tile([HID, 128], BF16, tag="h2t")
                nc.vector.tensor_copy(h2t[:], tp[:])
                ps2 = pspool.tile([128, NCLS], F32, tag="ps2")
                nc.tensor.matmul(ps2[:], lhsT=h2t[:], rhs=w2_sb[:],
                                 start=True, stop=True)
                ot = spool.tile([128, NCLS], F32, tag="ot")
                nc.vector.tensor_tensor(ot[:], ps2[:], b2_sb[:], ALU.add)
                nc.sync.dma_start(out.ap()[t * 128:(t + 1) * 128, :], ot[:])

            if stage >= 5:
                conv(tab2, dt1, bsc_sb[:, 0:1], bsc_sb[:, 1:2],
                     bsc_sb[:, 3:4], h_pr, ep_conv2)

    nc.compile()
    return nc


# ---------------------------------------------------------------- entry point

def kernel(x, W1, b1, W2, b2, beta2, edge_index):
    x = np.asarray(x, dtype=np.float32)
    W1 = np.asarray(W1, dtype=np.float32)
    b1 = np.asarray(b1, dtype=np.float32)
    W2 = np.asarray(W2, dtype=np.float32)
    b2 = np.asarray(b2, dtype=np.float32)
    beta2 = float(np.asarray(beta2))
    edge_index = np.asarray(edge_index)

    n = x.shape[0]
    g = _prep_graph(n, edge_index)
    tiles, nloc, nsh = g["tiles"], g["nloc"], g["nsh"]

    nc = _build(tiles, nloc, g["ks"], g["s_slots"],
                stage=int(os.environ.get("AGNN_STAGE", "99")))

    w1p = np.zeros((FPAD, HID), dtype=ml_dtypes.bfloat16)
    w1p[:FIN] = W1.astype(ml_dtypes.bfloat16)
    b1b = np.broadcast_to(b1, (128, HID)).copy().astype(np.float32)
    w2 = W2.astype(ml_dtypes.bfloat16)
    b2b = np.broadcast_to(b2, (128, NCLS)).copy().astype(np.float32)
    bscv = np.zeros((128, 4), dtype=np.float32)
    bscv[:, 0] = beta2
    bscv[:, 1] = -abs(beta2)
    bscv[:, 2] = 1.0                            # wself conv1 (beta=1)
    bscv[:, 3] = np.exp(beta2 - abs(beta2))     # wself conv2

    order = g["order"]
    xbf = x.astype(ml_dtypes.bfloat16)
    in_maps = []
    for c in range(NC):
        ids = order[c::NC]
        xs = np.zeros((FPAD, nloc), dtype=ml_dtypes.bfloat16)
        xs[:FIN, :len(ids)] = xbf[ids].T
        in_maps.append({
            "xT": xs, "W1": w1p, "b1b": b1b, "W2": w2, "b2b": b2b,
            "bsc": bscv,
            "offs": g["offs"][c], "msk": g["msk"][c], "msl": g["msl"][c],
        })

    trace = os.environ.get("AGNN_TRACE", "") == "1"
    kwargs = {}
    if trace:
        _enable_ntff_hook()
        import tempfile
        base = os.environ.get("AGNN_TRACE_DIR", "/tmp/agnn_traces")
        os.makedirs(base, exist_ok=True)
        kwargs = dict(trace=True, tmpdir=tempfile.mkdtemp(dir=base))
    res = run_bass_kernel_spmd(nc, in_maps, core_ids=list(range(NC)), **kwargs)
    if trace:
        print("AGNN exec_time_ns:", res.exec_time_ns)
        kernel._last_exec_time_ns = res.exec_time_ns

    out_full = np.empty((n, NCLS), dtype=np.float32)
    for c in range(NC):
        ids = order[c::NC]
        out_full[ids] = res.results[c]["out"][:len(ids)]
    return out_full


def _enable_ntff_hook():
    import antenv
    if "antenv.axon_hooks" not in sys.modules:
        mod = types.ModuleType("antenv.axon_hooks")
        _h = [None]
        mod.set_axon_ntff_profile_hook = lambda v: _h.__setitem__(0, v)
        mod.get_axon_ntff_profile_hook = lambda: _h[0]
        sys.modules["antenv.axon_hooks"] = mod
        antenv.axon_hooks = mod
    import concourse.bass_utils as bu
    bu.upload_artifacts = lambda d: d
    from trn_agent_boot.trn_boot import _ntff_profile_via_ctypes
    sys.modules["antenv.axon_hooks"].set_axon_ntff_profile_hook(
        _ntff_profile_via_ctypes("/opt/axon/libaxon_pjrt.so"))


# revision 17
# speedup vs baseline: 1.1937x; 1.0134x over previous
"""AGNN (2-layer attention GNN) distributed Bass kernel for 8 TRN2 NeuronCores.

Pipeline (per core, dst-sharded):
  h0 = relu(x @ W1 + b1)                      (node shard, x streamed bf16)
  conv1: h1[i] = sum_e softmax_e(cos(xn_s, xn_d)) * h0[src]
  conv2: same with beta2
  out = h2 @ W2 + b2

Graph prep on host: nodes degree-sorted, dealt round-robin to 8 cores;
per-dst padded slot tables (cross-core-uniform K_t per tile) drive an
indirect-DMA gather from an all-gathered bf16 payload table
[xn(32) | inv_norm(1) | pad(1)] per node.  Segment softmax uses the
self-loop bound (max logit == |beta|), so no segment-max pass is needed.
"""

import os
import sys
import types

sys.path.insert(0, "/opt/trn_rl_repo")

import numpy as np
import ml_dtypes

import concourse.bass as bass
import concourse.bacc as bacc
import concourse.tile as tile
import concourse.mybir as mybir
from concourse.bass import IndirectOffsetOnAxis
from concourse.bass_utils import run_bass_kernel_spmd
from concourse.masks import make_identity

F32 = mybir.dt.float32
BF16 = mybir.dt.bfloat16
I32 = mybir.dt.int32
AF = mybir.ActivationFunctionType
ALU = mybir.AluOpType
AX = mybir.AxisListType

NC = 8
FIN = 500
FPAD = 512
HID = 32
NCLS = 40
PW = 34          # payload row width (bf16): 32 xn + 1 invr + 1 pad
EPS_NORM = 1e-12
EPS_SM = 1e-16


# ---------------------------------------------------------------- host prep

def _prep_graph(n, edge_index):
    """Degree-sort nodes, deal round-robin to cores, build padded slot tables."""
    src = np.asarray(edge_index[0], dtype=np.int64)
    dst = np.asarray(edge_index[1], dtype=np.int64)
    keep = src != dst                           # self-loops handled on-chip
    src, dst = src[keep], dst[keep]
    e = src.shape[0]

    mself = np.bincount(np.asarray(edge_index[1])[
        np.asarray(edge_index[0]) == np.asarray(edge_index[1])], minlength=n)
    deg = np.bincount(dst, minlength=n)
    order = np.argsort(-deg, kind="stable")          # global rank -> node id
    rank = np.empty(n, dtype=np.int64)
    rank[order] = np.arange(n)

    nsh = -(-n // NC)                                 # nodes per core shard
    tiles = -(-nsh // 128)
    nloc = tiles * 128

    deg_sorted = deg[order]
    ks = []
    for t in range(tiles):
        w0 = t * 128 * NC
        k = int(deg_sorted[w0]) if w0 < n else 1
        ks.append(max(k, 1))
    ks = np.array(ks, dtype=np.int64)
    tile_base = np.concatenate([[0], np.cumsum(128 * ks)])
    s_slots = int(tile_base[-1])

    # slot index for each edge
    r_dst = rank[dst]
    eorder = np.argsort(r_dst, kind="stable")
    rs = r_dst[eorder]
    ssrc = src[eorder]
    cum = np.concatenate([[0], np.cumsum(deg_sorted)])
    k_in_dst = np.arange(e, dtype=np.int64) - cum[rs]

    core = rs % NC
    local = rs // NC
    t_of = local // 128
    p_of = local % 128
    j = tile_base[t_of] + p_of * ks[t_of] + k_in_dst

    # table position (payload row) of each node: owner core block + local rank
    tpos = (rank % NC) * nloc + rank // NC

    offs = np.zeros((NC, s_slots), dtype=np.int32)
    msk = np.zeros((NC, s_slots), dtype=np.float32)
    offs[core, j] = tpos[ssrc].astype(np.int32)
    msk[core, j] = 1.0

    msl = np.zeros((NC, 128, tiles), dtype=np.float32)
    cr = np.arange(n)
    msl[rank % NC, (rank // NC) % 128, (rank // NC) // 128] = mself[cr]
    return dict(order=order, rank=rank, nsh=nsh, tiles=tiles, nloc=nloc,
                ks=[int(k) for k in ks], s_slots=s_slots, offs=offs, msk=msk,
                msl=msl)


# ---------------------------------------------------------------- device build

def _build(tiles, nloc, ks, s_slots, stage=99):
    nc = bacc.Bacc("TRN2", target_bir_lowering=False, debug=False,
                   num_devices=NC)

    xT = nc.dram_tensor("xT", [FPAD, nloc], BF16, kind="ExternalInput")
    W1 = nc.dram_tensor("W1", [FPAD, HID], BF16, kind="ExternalInput")
    b1b = nc.dram_tensor("b1b", [128, HID], F32, kind="ExternalInput")
    W2 = nc.dram_tensor("W2", [HID, NCLS], BF16, kind="ExternalInput")
    b2b = nc.dram_tensor("b2b", [128, NCLS], F32, kind="ExternalInput")
    bsc = nc.dram_tensor("bsc", [128, 4], F32, kind="ExternalInput")
    offs = nc.dram_tensor("offs", [s_slots], I32, kind="ExternalInput")
    msl = nc.dram_tensor("msl", [128, tiles], F32, kind="ExternalInput")
    msk = nc.dram_tensor("msk", [s_slots], F32, kind="ExternalInput")
    out = nc.dram_tensor("out", [nloc, NCLS], F32, kind="ExternalOutput")

    kmax = max(ks)
    tile_base = [0]
    for k in ks:
        tile_base.append(tile_base[-1] + 128 * k)

    with tile.TileContext(nc) as tc:
        with tc.tile_pool(name="const", bufs=1) as cpool, \
             tc.tile_pool(name="persist", bufs=1) as ppool, \
             tc.tile_pool(name="xio", bufs=3) as xpool, \
             tc.tile_pool(name="gio", bufs=4) as gpool, \
             tc.tile_pool(name="work", bufs=4) as wpool, \
             tc.tile_pool(name="small", bufs=4) as spool, \
             tc.tile_pool(name="ps", bufs=2, space="PSUM") as pspool, \
             tc.tile_pool(name="dram", bufs=1, space="DRAM") as dpool:

            # ---- constants to SBUF
            w1_sb = cpool.tile([128, 4, HID], BF16)
            nc.sync.dma_start(w1_sb[:], W1.ap().rearrange("(c p) h -> p c h", p=128))
            b1_sb = cpool.tile([128, HID], F32)
            nc.sync.dma_start(b1_sb[:], b1b.ap())
            w2_sb = cpool.tile([HID, NCLS], BF16)
            nc.sync.dma_start(w2_sb[:], W2.ap())
            b2_sb = cpool.tile([128, NCLS], F32)
            nc.sync.dma_start(b2_sb[:], b2b.ap())
            bsc_sb = cpool.tile([128, 4], F32)
            nc.sync.dma_start(bsc_sb[:], bsc.ap())
            msl_sb = cpool.tile([128, tiles], F32)
            nc.sync.dma_start(msl_sb[:], msl.ap())
            ident = cpool.tile([128, 128], F32)
            make_identity(nc, ident[:])
            c_eps = cpool.tile([128, 1], F32)
            nc.gpsimd.memset(c_eps[:], EPS_NORM)
            c_neg1 = cpool.tile([128, 1], F32)
            nc.gpsimd.memset(c_neg1[:], -1.0)

            # ---- persistent node state
            h_sb = ppool.tile([128, tiles, HID], F32)     # current features
            h_pr = ppool.tile([128, tiles, HID], F32)     # prev-layer features
            dt0 = ppool.tile([128, tiles, HID], BF16)     # xn for conv1 dst
            dt1 = ppool.tile([128, tiles, HID], BF16)     # xn for conv2 dst

            pay1 = dpool.tile([nloc, PW], BF16)
            pay2 = dpool.tile([nloc, PW], BF16)
            tab1 = dpool.tile([NC * nloc, PW], BF16)
            tab2 = dpool.tile([NC * nloc, PW], BF16)

            xT_re = xT.ap().rearrange("(c p) n -> p c n", p=128)

            def normalize_payload(h_ap, dt_ap, pay_dram, t, sub=99):
                """xn = h/|h|; payload row [xn | invr | junk] -> pay_dram."""
                hsq = spool.tile([128, HID], F32, tag="hsq")
                ss = spool.tile([128, 1], F32, tag="ss")
                nc.vector.tensor_tensor(hsq[:], h_ap, h_ap, ALU.mult)
                nc.vector.tensor_reduce(ss[:], hsq[:], AX.X, ALU.add)
                if sub <= 1:
                    nc.vector.tensor_copy(dt_ap, h_ap)
                    return
                lss = spool.tile([128, 1], F32, tag="lss")
                nc.scalar.activation(lss[:], ss[:], AF.Ln, bias=c_eps[:])
                invr = spool.tile([128, 1], F32, tag="invr")
                nc.scalar.activation(invr[:], lss[:], AF.Exp, scale=-0.5)
                rr = spool.tile([128, 1], F32, tag="rr")
                nc.scalar.activation(rr[:], lss[:], AF.Exp, scale=0.5)
                if sub <= 2:
                    nc.vector.tensor_copy(dt_ap, h_ap)
                    return
                xn = spool.tile([128, HID], F32, tag="xn")
                nc.vector.tensor_scalar_mul(xn[:], h_ap, invr[:])
                payt = spool.tile([128, PW], BF16, tag="payt")
                nc.vector.tensor_copy(payt[:, 0:HID], xn[:])
                nc.vector.tensor_copy(payt[:, HID:HID + 2],
                                      rr[:].to_broadcast([128, 2]))
                nc.vector.tensor_copy(dt_ap, xn[:])
                if sub <= 3:
                    return
                nc.sync.dma_start(pay_dram[t * 128:(t + 1) * 128, :],
                                   payt[:])

            def dump_h(t, ap32, cast=False):
                ot = spool.tile([128, NCLS], F32, tag="ot_dbg")
                nc.gpsimd.memset(ot[:], 0.0)
                nc.vector.tensor_copy(ot[:, 0:HID], ap32)
                nc.sync.dma_start(out.ap()[t * 128:(t + 1) * 128, :], ot[:])

            # ================= layer 1 + payload for conv1
            for t in range(tiles):
                xt = xpool.tile([128, 4, 128], BF16, tag="xt")
                nc.sync.dma_start(xt[:], xT_re[:, :, t * 128:(t + 1) * 128])
                ps = pspool.tile([128, HID], F32, tag="ps1")
                for c in range(4):
                    nc.tensor.matmul(ps[:], lhsT=xt[:, c, :], rhs=w1_sb[:, c, :],
                                     start=(c == 0), stop=(c == 3))
                h_t = h_sb[:, t, :]
                nc.vector.tensor_tensor(h_t, ps[:], b1_sb[:], ALU.add)
                nc.vector.tensor_relu(h_t, h_t)
                if stage == 1:
                    dump_h(t, h_t)
                    continue
                if 11 <= stage <= 15:
                    normalize_payload(h_t, dt0[:, t, :], pay1, t, sub=stage - 11)
                    nc.vector.tensor_copy(dt1[:, t, :], h_t)
                    dump_h(t, h_t)
                    continue
                normalize_payload(h_t, dt0[:, t, :], pay1, t)

            if stage == 1:
                nc.compile_marker = True
            if stage >= 2 and not (11 <= stage <= 15):
                nc.gpsimd.collective_compute(
                "AllGather", ALU.bypass,
                    replica_groups=[list(range(NC))],
                    ins=[pay1.opt()], outs=[tab1.opt()])
            if stage == 2:
                for t in range(tiles):
                    dump_h(t, h_sb[:, t, :])

            # ================= conv layer
            def conv(table, dts, scale_arg, bias_arg, wself_arg, h_prev, epilogue):
                for t in range(tiles):
                    k = ks[t]
                    base = tile_base[t]
                    off_t = gpool.tile([128, kmax], I32, tag="off")
                    nc.sync.dma_start(
                        off_t[:, 0:k],
                        offs.ap()[base:base + 128 * k]
                            .rearrange("(p k) -> p k", k=k))
                    msk_t = gpool.tile([128, kmax], F32, tag="msk")
                    nc.sync.dma_start(
                        msk_t[:, 0:k],
                        msk.ap()[base:base + 128 * k]
                            .rearrange("(p k) -> p k", k=k))
                    g = gpool.tile([128, kmax * PW], BF16, tag="g")
                    for kk in range(k):
                        nc.gpsimd.indirect_dma_start(
                            out=g[:, kk * PW:(kk + 1) * PW], out_offset=None,
                            in_=table[:],
                            in_offset=IndirectOffsetOnAxis(
                                ap=off_t[:, kk:kk + 1], axis=0))
                    g3 = g[:, 0:k * PW].rearrange("p (k d) -> p k d", d=PW)

                    if stage == 3:
                        agg0 = spool.tile([128, HID], F32, tag="agg")
                        nc.vector.tensor_reduce(
                            agg0[:],
                            g[:, 0:k * PW].rearrange("p (k d) -> p d k", d=PW)[:, 0:HID, :],
                            AX.X, ALU.add)
                        dump_h(t, agg0[:])
                        continue

                    pl = wpool.tile([128, kmax * HID], BF16, tag="pl")
                    pl3 = pl[:, 0:k * HID].rearrange("p (k f) -> p k f",
                                                          f=HID)
                    dbc = dts[:, t, :].unsqueeze(1).to_broadcast([128, k, HID])
                    nc.vector.tensor_tensor(pl3, g3[:, :, 0:HID], dbc, ALU.mult)

                    ll = wpool.tile([128, kmax], F32, tag="ll")
                    nc.vector.tensor_reduce(ll[:, 0:k], pl3, AX.X, ALU.add)

                    w = wpool.tile([128, kmax], F32, tag="w")
                    nc.scalar.activation(w[:, 0:k], ll[:, 0:k], AF.Exp,
                                         bias=bias_arg, scale=scale_arg)
                    wm = wpool.tile([128, kmax], F32, tag="wm")
                    nc.vector.tensor_tensor(wm[:, 0:k], w[:, 0:k], msk_t[:, 0:k],
                                            ALU.mult)
                    z = spool.tile([128, 1], F32, tag="z")
                    nc.vector.tensor_reduce(z[:], wm[:, 0:k], AX.X, ALU.add)
                    nc.vector.tensor_scalar_add(z[:], z[:], EPS_SM)
                    # self-loop: z += wself * multiplicity
                    zs = spool.tile([128, 1], F32, tag="zs")
                    nc.vector.tensor_tensor(zs[:], msl_sb[:, t:t + 1],
                                            wself_arg, ALU.mult)
                    nc.vector.tensor_tensor(z[:], z[:], zs[:], ALU.add)
                    rz = spool.tile([128, 1], F32, tag="rz")
                    nc.vector.reciprocal(rz[:], z[:])

                    invc = wpool.tile([128, kmax], F32, tag="invc")
                    nc.vector.tensor_copy(invc[:, 0:k], g3[:, :, HID:HID + 1]
                                          .rearrange("p k d -> p (k d)"))
                    wp = wpool.tile([128, kmax], BF16, tag="wp")
                    nc.vector.scalar_tensor_tensor(
                        out=wp[:, 0:k], in0=wm[:, 0:k], scalar=1.0,
                        in1=invc[:, 0:k], op0=ALU.mult, op1=ALU.mult)

                    pa = wpool.tile([128, kmax * HID], BF16, tag="pa")
                    pa3 = pa[:, 0:k * HID].rearrange("p (k f) -> p k f",
                                                          f=HID)
                    wbc = wp[:, 0:k].unsqueeze(2).to_broadcast([128, k, HID])
                    nc.vector.tensor_tensor(pa3, g3[:, :, 0:HID], wbc, ALU.mult)

                    agg = spool.tile([128, HID], F32, tag="agg")
                    nc.vector.tensor_reduce(
                        agg[:],
                        pa[:, 0:k * HID].rearrange("p (k f) -> p f k", f=HID),
                        AX.X, ALU.add)
                    # self-loop: agg += wself * h_prev[tile]
                    hw = spool.tile([128, HID], F32, tag="hwself")
                    nc.vector.tensor_scalar_mul(hw[:], h_prev[:, t, :], zs[:])
                    nc.vector.tensor_add(agg[:], agg[:], hw[:])
                    h_new = spool.tile([128, HID], F32, tag="hnew")
                    nc.vector.tensor_scalar_mul(h_new[:], agg[:], rz[:])
                    epilogue(t, h_new)

            def ep_conv1(t, h_new):
                nc.vector.tensor_copy(h_pr[:, t, :], h_new[:])
                if stage == 4:
                    dump_h(t, h_new[:])
                    return
                normalize_payload(h_pr[:, t, :], dt1[:, t, :], pay2, t)

            if stage >= 3 and not (11 <= stage <= 15):
                conv(tab1, dt0, 1.0, c_neg1[:], bsc_sb[:, 2:3], h_sb, ep_conv1)

            if stage >= 5:
                nc.gpsimd.collective_compute(
                "AllGather", ALU.bypass,
                    replica_groups=[list(range(NC))],
                    ins=[pay2.opt()], outs=[tab2.opt()])

            def ep_conv2(t, h_new):
                tp = pspool.tile([HID, 128], F32, tag="tp")
                nc.tensor.transpose(out=tp[:], in_=h_new[:], identity=ident[:])
                h2t = spool.tile([HID, 128], BF16, tag="h2t")
                nc.vector.tensor_copy(h2t[:], tp[:])
                ps2 = pspool.tile([128, NCLS], F32, tag="ps2")
                nc.tensor.matmul(ps2[:], lhsT=h2t[:], rhs=w2_sb[:],
                                 start=True, stop=True)
                ot = spool.tile([128, NCLS], F32, tag="ot")
                nc.vector.tensor_tensor(ot[:], ps2[:], b2_sb[:], ALU.add)
                nc.sync.dma_start(out.ap()[t * 128:(t + 1) * 128, :], ot[:])

            if stage >= 5:
                conv(tab2, dt1, bsc_sb[:, 0:1], bsc_sb[:, 1:2],
                     bsc_sb[:, 3:4], h_pr, ep_conv2)

    nc.compile()
    return nc


# ---------------------------------------------------------------- entry point

def kernel(x, W1, b1, W2, b2, beta2, edge_index):
    x = np.asarray(x, dtype=np.float32)
    W1 = np.asarray(W1, dtype=np.float32)
    b1 = np.asarray(b1, dtype=np.float32)
    W2 = np.asarray(W2, dtype=np.float32)
    b2 = np.asarray(b2, dtype=np.float32)
    beta2 = float(np.asarray(beta2))
    edge_index = np.asarray(edge_index)

    n = x.shape[0]
    g = _prep_graph(n, edge_index)
    tiles, nloc, nsh = g["tiles"], g["nloc"], g["nsh"]

    nc = _build(tiles, nloc, g["ks"], g["s_slots"],
                stage=int(os.environ.get("AGNN_STAGE", "99")))

    w1p = np.zeros((FPAD, HID), dtype=ml_dtypes.bfloat16)
    w1p[:FIN] = W1.astype(ml_dtypes.bfloat16)
    b1b = np.broadcast_to(b1, (128, HID)).copy().astype(np.float32)
    w2 = W2.astype(ml_dtypes.bfloat16)
    b2b = np.broadcast_to(b2, (128, NCLS)).copy().astype(np.float32)
    bscv = np.zeros((128, 4), dtype=np.float32)
    bscv[:, 0] = beta2
    bscv[:, 1] = -abs(beta2)
    bscv[:, 2] = 1.0                            # wself conv1 (beta=1)
    bscv[:, 3] = np.exp(beta2 - abs(beta2))     # wself conv2

    order = g["order"]
    xbf = x.astype(ml_dtypes.bfloat16)
    in_maps = []
    for c in range(NC):
        ids = order[c::NC]
        xs = np.zeros((FPAD, nloc), dtype=ml_dtypes.bfloat16)
        xs[:FIN, :len(ids)] = xbf[ids].T
        in_maps.append({
            "xT": xs, "W1": w1p, "b1b": b1b, "W2": w2, "b2b": b2b,
            "bsc": bscv,
            "offs": g["offs"][c], "msk": g["msk"][c], "msl": g["msl"][c],
        })

    trace = os.environ.get("AGNN_TRACE", "") == "1"
    kwargs = {}
    if trace:
        _enable_ntff_hook()
        import tempfile
        base = os.environ.get("AGNN_TRACE_DIR", "/tmp/agnn_traces")
        os.makedirs(base, exist_ok=True)
        kwargs = dict(trace=True, tmpdir=tempfile.mkdtemp(dir=base))
    res = run_bass_kernel_spmd(nc, in_maps, core_ids=list(range(NC)), **kwargs)
    if trace:
        print("AGNN exec_time_ns:", res.exec_time_ns)
        kernel._last_exec_time_ns = res.exec_time_ns

    out_full = np.empty((n, NCLS), dtype=np.float32)
    for c in range(NC):
        ids = order[c::NC]
        out_full[ids] = res.results[c]["out"][:len(ids)]
    return out_full


def _enable_ntff_hook():
    import antenv
    if "antenv.axon_hooks" not in sys.modules:
        mod = types.ModuleType("antenv.axon_hooks")
        _h = [None]
        mod.set_axon_ntff_profile_hook = lambda v: _h.__setitem__(0, v)
        mod.get_axon_ntff_profile_hook = lambda: _h[0]
        sys.modules["antenv.axon_hooks"] = mod
        antenv.axon_hooks = mod
    import concourse.bass_utils as bu
    bu.upload_artifacts = lambda d: d
    from trn_agent_boot.trn_boot import _ntff_profile_via_ctypes
    sys.modules["antenv.axon_hooks"].set_axon_ntff_profile_hook(
        _ntff_profile_via_ctypes("/opt/axon/libaxon_pjrt.so"))


# revision 19
# speedup vs baseline: 1.1966x; 1.0025x over previous
"""AGNN (2-layer attention GNN) distributed Bass kernel for 8 TRN2 NeuronCores.

Pipeline (per core, dst-sharded):
  h0 = relu(x @ W1 + b1)                      (node shard, x streamed bf16)
  conv1: h1[i] = sum_e softmax_e(cos(xn_s, xn_d)) * h0[src]
  conv2: same with beta2
  out = h2 @ W2 + b2

Graph prep on host: nodes degree-sorted, dealt round-robin to 8 cores;
per-dst padded slot tables (cross-core-uniform K_t per tile) drive an
indirect-DMA gather from an all-gathered bf16 payload table
[xn(32) | inv_norm(1) | pad(1)] per node.  Segment softmax uses the
self-loop bound (max logit == |beta|), so no segment-max pass is needed.
"""

import os
import sys
import types

sys.path.insert(0, "/opt/trn_rl_repo")

import numpy as np
import ml_dtypes

import concourse.bass as bass
import concourse.bacc as bacc
import concourse.tile as tile
import concourse.mybir as mybir
from concourse.bass import IndirectOffsetOnAxis
from concourse.bass_utils import run_bass_kernel_spmd
from concourse.masks import make_identity

F32 = mybir.dt.float32
BF16 = mybir.dt.bfloat16
I32 = mybir.dt.int32
AF = mybir.ActivationFunctionType
ALU = mybir.AluOpType
AX = mybir.AxisListType

NC = 8
FIN = 500
FPAD = 512
HID = 32
NCLS = 40
PW = 34          # payload row width (bf16): 32 xn + 1 invr + 1 pad
EPS_NORM = 1e-12
EPS_SM = 1e-16


# ---------------------------------------------------------------- host prep

def _prep_graph(n, edge_index):
    """Degree-sort nodes, deal round-robin to cores, build padded slot tables."""
    src = np.asarray(edge_index[0], dtype=np.int64)
    dst = np.asarray(edge_index[1], dtype=np.int64)
    keep = src != dst                           # self-loops handled on-chip
    src, dst = src[keep], dst[keep]
    e = src.shape[0]

    mself = np.bincount(np.asarray(edge_index[1])[
        np.asarray(edge_index[0]) == np.asarray(edge_index[1])], minlength=n)
    deg = np.bincount(dst, minlength=n)
    order = np.argsort(-deg, kind="stable")          # global rank -> node id
    rank = np.empty(n, dtype=np.int64)
    rank[order] = np.arange(n)

    nsh = -(-n // NC)                                 # nodes per core shard
    tiles = -(-nsh // 128)
    nloc = tiles * 128

    deg_sorted = deg[order]
    ks = []
    for t in range(tiles):
        w0 = t * 128 * NC
        k = int(deg_sorted[w0]) if w0 < n else 1
        ks.append(max(k, 1))
    ks = np.array(ks, dtype=np.int64)
    tile_base = np.concatenate([[0], np.cumsum(128 * ks)])
    s_slots = int(tile_base[-1])

    # slot index for each edge
    r_dst = rank[dst]
    eorder = np.argsort(r_dst, kind="stable")
    rs = r_dst[eorder]
    ssrc = src[eorder]
    cum = np.concatenate([[0], np.cumsum(deg_sorted)])
    k_in_dst = np.arange(e, dtype=np.int64) - cum[rs]

    core = rs % NC
    local = rs // NC
    t_of = local // 128
    p_of = local % 128
    j = tile_base[t_of] + p_of * ks[t_of] + k_in_dst

    # table position (payload row) of each node: owner core block + local rank
    tpos = (rank % NC) * nloc + rank // NC

    offs = np.zeros((NC, s_slots), dtype=np.int32)
    msk = np.zeros((NC, s_slots), dtype=np.float32)
    offs[core, j] = tpos[ssrc].astype(np.int32)
    msk[core, j] = 1.0

    msl = np.zeros((NC, 128, tiles), dtype=np.float32)
    cr = np.arange(n)
    msl[rank % NC, (rank // NC) % 128, (rank // NC) // 128] = mself[cr]
    return dict(order=order, rank=rank, nsh=nsh, tiles=tiles, nloc=nloc,
                ks=[int(k) for k in ks], s_slots=s_slots, offs=offs, msk=msk,
                msl=msl)


# ---------------------------------------------------------------- device build

def _build(tiles, nloc, ks, s_slots, stage=99):
    nc = bacc.Bacc("TRN2", target_bir_lowering=False, debug=False,
                   num_devices=NC)

    xT = nc.dram_tensor("xT", [FPAD, nloc], BF16, kind="ExternalInput")
    W1 = nc.dram_tensor("W1", [FPAD, HID], BF16, kind="ExternalInput")
    b1b = nc.dram_tensor("b1b", [128, HID], F32, kind="ExternalInput")
    W2 = nc.dram_tensor("W2", [HID, NCLS], BF16, kind="ExternalInput")
    b2b = nc.dram_tensor("b2b", [128, NCLS], F32, kind="ExternalInput")
    bsc = nc.dram_tensor("bsc", [128, 4], F32, kind="ExternalInput")
    offs = nc.dram_tensor("offs", [s_slots], I32, kind="ExternalInput")
    msl = nc.dram_tensor("msl", [128, tiles], F32, kind="ExternalInput")
    msk = nc.dram_tensor("msk", [s_slots], F32, kind="ExternalInput")
    out = nc.dram_tensor("out", [nloc, NCLS], F32, kind="ExternalOutput")

    kmax = max(ks)
    tile_base = [0]
    for k in ks:
        tile_base.append(tile_base[-1] + 128 * k)

    with tile.TileContext(nc) as tc:
        with tc.tile_pool(name="const", bufs=1) as cpool, \
             tc.tile_pool(name="persist", bufs=1) as ppool, \
             tc.tile_pool(name="xio", bufs=3) as xpool, \
             tc.tile_pool(name="gio", bufs=4) as gpool, \
             tc.tile_pool(name="work", bufs=4) as wpool, \
             tc.tile_pool(name="small", bufs=4) as spool, \
             tc.tile_pool(name="ps", bufs=2, space="PSUM") as pspool, \
             tc.tile_pool(name="dram", bufs=1, space="DRAM") as dpool:

            # ---- constants to SBUF
            w1_sb = cpool.tile([128, 4, HID], BF16)
            nc.sync.dma_start(w1_sb[:], W1.ap().rearrange("(c p) h -> p c h", p=128))
            b1_sb = cpool.tile([128, HID], F32)
            nc.sync.dma_start(b1_sb[:], b1b.ap())
            w2_sb = cpool.tile([HID, NCLS], BF16)
            nc.sync.dma_start(w2_sb[:], W2.ap())
            b2_sb = cpool.tile([128, NCLS], F32)
            nc.sync.dma_start(b2_sb[:], b2b.ap())
            bsc_sb = cpool.tile([128, 4], F32)
            nc.sync.dma_start(bsc_sb[:], bsc.ap())
            msl_sb = cpool.tile([128, tiles], F32)
            nc.sync.dma_start(msl_sb[:], msl.ap())
            ident = cpool.tile([128, 128], F32)
            make_identity(nc, ident[:])
            c_eps = cpool.tile([128, 1], F32)
            nc.gpsimd.memset(c_eps[:], EPS_NORM)
            c_neg1 = cpool.tile([128, 1], F32)
            nc.gpsimd.memset(c_neg1[:], -1.0)

            # ---- persistent node state
            h_sb = ppool.tile([128, tiles, HID], F32)     # current features
            h_pr = ppool.tile([128, tiles, HID], F32)     # prev-layer features
            dt0 = ppool.tile([128, tiles, HID], BF16)     # xn for conv1 dst
            dt1 = ppool.tile([128, tiles, HID], BF16)     # xn for conv2 dst

            pay1 = dpool.tile([nloc, PW], BF16)
            pay2 = dpool.tile([nloc, PW], BF16)
            tab1 = dpool.tile([NC * nloc, PW], BF16)
            tab2 = dpool.tile([NC * nloc, PW], BF16)

            xT_re = xT.ap().rearrange("(c p) n -> p c n", p=128)

            def normalize_payload(h_ap, dt_ap, pay_dram, t, sub=99):
                """xn = h/|h|; payload row [xn | invr | junk] -> pay_dram."""
                hsq = spool.tile([128, HID], F32, tag="hsq")
                ss = spool.tile([128, 1], F32, tag="ss")
                nc.vector.tensor_tensor(hsq[:], h_ap, h_ap, ALU.mult)
                nc.vector.tensor_reduce(ss[:], hsq[:], AX.X, ALU.add)
                if sub <= 1:
                    nc.vector.tensor_copy(dt_ap, h_ap)
                    return
                lss = spool.tile([128, 1], F32, tag="lss")
                nc.scalar.activation(lss[:], ss[:], AF.Ln, bias=c_eps[:])
                invr = spool.tile([128, 1], F32, tag="invr")
                nc.scalar.activation(invr[:], lss[:], AF.Exp, scale=-0.5)
                rr = spool.tile([128, 1], F32, tag="rr")
                nc.scalar.activation(rr[:], lss[:], AF.Exp, scale=0.5)
                if sub <= 2:
                    nc.vector.tensor_copy(dt_ap, h_ap)
                    return
                xn = spool.tile([128, HID], F32, tag="xn")
                nc.vector.tensor_scalar_mul(xn[:], h_ap, invr[:])
                payt = spool.tile([128, PW], BF16, tag="payt")
                nc.vector.tensor_copy(payt[:, 0:HID], xn[:])
                nc.vector.tensor_copy(payt[:, HID:HID + 2],
                                      rr[:].to_broadcast([128, 2]))
                nc.vector.tensor_copy(dt_ap, xn[:])
                if sub <= 3:
                    return
                nc.sync.dma_start(pay_dram[t * 128:(t + 1) * 128, :],
                                   payt[:])

            def dump_h(t, ap32, cast=False):
                ot = spool.tile([128, NCLS], F32, tag="ot_dbg")
                nc.gpsimd.memset(ot[:], 0.0)
                nc.vector.tensor_copy(ot[:, 0:HID], ap32)
                nc.sync.dma_start(out.ap()[t * 128:(t + 1) * 128, :], ot[:])

            # ================= layer 1 + payload for conv1
            for t in range(tiles):
                xt = xpool.tile([128, 4, 128], BF16, tag="xt")
                nc.sync.dma_start(xt[:], xT_re[:, :, t * 128:(t + 1) * 128])
                ps = pspool.tile([128, HID], F32, tag="ps1")
                for c in range(4):
                    nc.tensor.matmul(ps[:], lhsT=xt[:, c, :], rhs=w1_sb[:, c, :],
                                     start=(c == 0), stop=(c == 3))
                h_t = h_sb[:, t, :]
                nc.vector.tensor_tensor(h_t, ps[:], b1_sb[:], ALU.add)
                nc.vector.tensor_relu(h_t, h_t)
                if stage == 1:
                    dump_h(t, h_t)
                    continue
                if 11 <= stage <= 15:
                    normalize_payload(h_t, dt0[:, t, :], pay1, t, sub=stage - 11)
                    nc.vector.tensor_copy(dt1[:, t, :], h_t)
                    dump_h(t, h_t)
                    continue
                normalize_payload(h_t, dt0[:, t, :], pay1, t)

            if stage == 1:
                nc.compile_marker = True
            if stage >= 2 and not (11 <= stage <= 15):
                nc.gpsimd.collective_compute(
                "AllGather", ALU.bypass,
                    replica_groups=[list(range(NC))],
                    ins=[pay1.opt()], outs=[tab1.opt()])
            if stage == 2:
                for t in range(tiles):
                    dump_h(t, h_sb[:, t, :])

            # ================= conv layer
            def conv(table, dts, scale_arg, bias_arg, wself_arg, h_prev, epilogue):
                for t in range(tiles):
                    k = ks[t]
                    base = tile_base[t]
                    off_t = gpool.tile([128, kmax], I32, tag="off")
                    nc.sync.dma_start(
                        off_t[:, 0:k],
                        offs.ap()[base:base + 128 * k]
                            .rearrange("(p k) -> p k", k=k))
                    msk_t = gpool.tile([128, kmax], F32, tag="msk")
                    nc.sync.dma_start(
                        msk_t[:, 0:k],
                        msk.ap()[base:base + 128 * k]
                            .rearrange("(p k) -> p k", k=k))
                    g = gpool.tile([128, kmax * PW], BF16, tag="g")
                    for kk in range(k):
                        nc.gpsimd.indirect_dma_start(
                            out=g[:, kk * PW:(kk + 1) * PW], out_offset=None,
                            in_=table[:],
                            in_offset=IndirectOffsetOnAxis(
                                ap=off_t[:, kk:kk + 1], axis=0))
                    g3 = g[:, 0:k * PW].rearrange("p (k d) -> p k d", d=PW)

                    if stage == 3:
                        agg0 = spool.tile([128, HID], F32, tag="agg")
                        nc.vector.tensor_reduce(
                            agg0[:],
                            g[:, 0:k * PW].rearrange("p (k d) -> p d k", d=PW)[:, 0:HID, :],
                            AX.X, ALU.add)
                        dump_h(t, agg0[:])
                        continue

                    pl = wpool.tile([128, kmax * HID], BF16, tag="pl")
                    pl3 = pl[:, 0:k * HID].rearrange("p (k f) -> p k f",
                                                          f=HID)
                    dbc = dts[:, t, :].unsqueeze(1).to_broadcast([128, k, HID])
                    nc.vector.tensor_tensor(pl3, g3[:, :, 0:HID], dbc, ALU.mult)

                    ll = wpool.tile([128, kmax], F32, tag="ll")
                    nc.vector.tensor_reduce(ll[:, 0:k], pl3, AX.X, ALU.add)

                    w = wpool.tile([128, kmax], F32, tag="w")
                    nc.scalar.activation(w[:, 0:k], ll[:, 0:k], AF.Exp,
                                         bias=bias_arg, scale=scale_arg)
                    wm = wpool.tile([128, kmax], F32, tag="wm")
                    nc.vector.tensor_tensor(wm[:, 0:k], w[:, 0:k], msk_t[:, 0:k],
                                            ALU.mult)
                    z = spool.tile([128, 1], F32, tag="z")
                    nc.vector.tensor_reduce(z[:], wm[:, 0:k], AX.X, ALU.add)
                    nc.vector.tensor_scalar_add(z[:], z[:], EPS_SM)
                    # self-loop: z += wself * multiplicity
                    zs = spool.tile([128, 1], F32, tag="zs")
                    nc.vector.tensor_tensor(zs[:], msl_sb[:, t:t + 1],
                                            wself_arg, ALU.mult)
                    nc.vector.tensor_tensor(z[:], z[:], zs[:], ALU.add)
                    rz = spool.tile([128, 1], F32, tag="rz")
                    nc.vector.reciprocal(rz[:], z[:])

                    invc = wpool.tile([128, kmax], F32, tag="invc")
                    nc.vector.tensor_copy(invc[:, 0:k], g3[:, :, HID:HID + 1]
                                          .rearrange("p k d -> p (k d)"))
                    wp = wpool.tile([128, kmax], BF16, tag="wp")
                    nc.vector.scalar_tensor_tensor(
                        out=wp[:, 0:k], in0=wm[:, 0:k], scalar=1.0,
                        in1=invc[:, 0:k], op0=ALU.mult, op1=ALU.mult)

                    pa = wpool.tile([128, kmax * HID], BF16, tag="pa")
                    pa3 = pa[:, 0:k * HID].rearrange("p (k f) -> p k f",
                                                          f=HID)
                    wbc = wp[:, 0:k].unsqueeze(2).to_broadcast([128, k, HID])
                    nc.vector.tensor_tensor(pa3, g3[:, :, 0:HID], wbc, ALU.mult)

                    agg = spool.tile([128, HID], F32, tag="agg")
                    nc.vector.tensor_reduce(
                        agg[:],
                        pa[:, 0:k * HID].rearrange("p (k f) -> p f k", f=HID),
                        AX.X, ALU.add)
                    # self-loop: agg += wself * h_prev[tile]
                    hw = spool.tile([128, HID], F32, tag="hwself")
                    nc.vector.tensor_scalar_mul(hw[:], h_prev[:, t, :], zs[:])
                    nc.vector.tensor_add(agg[:], agg[:], hw[:])
                    h_new = spool.tile([128, HID], F32, tag="hnew")
                    nc.vector.tensor_scalar_mul(h_new[:], agg[:], rz[:])
                    epilogue(t, h_new)

            def ep_conv1(t, h_new):
                nc.vector.tensor_copy(h_pr[:, t, :], h_new[:])
                if stage == 4:
                    dump_h(t, h_new[:])
                    return
                normalize_payload(h_pr[:, t, :], dt1[:, t, :], pay2, t)

            if stage >= 3 and not (11 <= stage <= 15):
                conv(tab1, dt0, 1.0, c_neg1[:], bsc_sb[:, 2:3], h_sb, ep_conv1)

            if stage >= 5:
                nc.gpsimd.collective_compute(
                "AllGather", ALU.bypass,
                    replica_groups=[list(range(NC))],
                    ins=[pay2.opt()], outs=[tab2.opt()])

            def ep_conv2(t, h_new):
                tp = pspool.tile([HID, 128], F32, tag="tp")
                nc.tensor.transpose(out=tp[:], in_=h_new[:], identity=ident[:])
                h2t = spool.tile([HID, 128], BF16, tag="h2t")
                nc.vector.tensor_copy(h2t[:], tp[:])
                ps2 = pspool.tile([128, NCLS], F32, tag="ps2")
                nc.tensor.matmul(ps2[:], lhsT=h2t[:], rhs=w2_sb[:],
                                 start=True, stop=True)
                ot = spool.tile([128, NCLS], F32, tag="ot")
                nc.vector.tensor_tensor(ot[:], ps2[:], b2_sb[:], ALU.add)
                nc.sync.dma_start(out.ap()[t * 128:(t + 1) * 128, :], ot[:])

            if stage >= 5:
                conv(tab2, dt1, bsc_sb[:, 0:1], bsc_sb[:, 1:2],
                     bsc_sb[:, 3:4], h_pr, ep_conv2)

    nc.compile()
    return nc


# ---------------------------------------------------------------- entry point

def kernel(x, W1, b1, W2, b2, beta2, edge_index):
    x = np.asarray(x, dtype=np.float32)
    W1 = np.asarray(W1, dtype=np.float32)
    b1 = np.asarray(b1, dtype=np.float32)
    W2 = np.asarray(W2, dtype=np.float32)
    b2 = np.asarray(b2, dtype=np.float32)
    beta2 = float(np.asarray(beta2))
    edge_index = np.asarray(edge_index)

    n = x.shape[0]
    g = _prep_graph(n, edge_index)
    tiles, nloc, nsh = g["tiles"], g["nloc"], g["nsh"]

    nc = _build(tiles, nloc, g["ks"], g["s_slots"],
                stage=int(os.environ.get("AGNN_STAGE", "99")))

    w1p = np.zeros((FPAD, HID), dtype=ml_dtypes.bfloat16)
    w1p[:FIN] = W1.astype(ml_dtypes.bfloat16)
    b1b = np.broadcast_to(b1, (128, HID)).copy().astype(np.float32)
    w2 = W2.astype(ml_dtypes.bfloat16)
    b2b = np.broadcast_to(b2, (128, NCLS)).copy().astype(np.float32)
    bscv = np.zeros((128, 4), dtype=np.float32)
    bscv[:, 0] = beta2
    bscv[:, 1] = -abs(beta2)
    bscv[:, 2] = 1.0                            # wself conv1 (beta=1)
    bscv[:, 3] = np.exp(beta2 - abs(beta2))     # wself conv2

    order = g["order"]
    xbf = x.astype(ml_dtypes.bfloat16)
    in_maps = []
    for c in range(NC):
        ids = order[c::NC]
        xs = np.zeros((FPAD, nloc), dtype=ml_dtypes.bfloat16)
        xs[:FIN, :len(ids)] = xbf[ids].T
        in_maps.append({
            "xT": xs, "W1": w1p, "b1b": b1b, "W2": w2, "b2b": b2b,
            "bsc": bscv,
            "offs": g["offs"][c], "msk": g["msk"][c], "msl": g["msl"][c],
        })

    trace = os.environ.get("AGNN_TRACE", "") == "1"
    kwargs = {}
    if trace:
        _enable_ntff_hook()
        import tempfile
        base = os.environ.get("AGNN_TRACE_DIR", "/tmp/agnn_traces")
        os.makedirs(base, exist_ok=True)
        kwargs = dict(trace=True, tmpdir=tempfile.mkdtemp(dir=base))
    res = run_bass_kernel_spmd(nc, in_maps, core_ids=list(range(NC)), **kwargs)
    if trace:
        print("AGNN exec_time_ns:", res.exec_time_ns)
        kernel._last_exec_time_ns = res.exec_time_ns

    out_full = np.empty((n, NCLS), dtype=np.float32)
    for c in range(NC):
        ids = order[c::NC]
        out_full[ids] = res.results[c]["out"][:len(ids)]
    return out_full


def _enable_ntff_hook():
    import antenv
    if "antenv.axon_hooks" not in sys.modules:
        mod = types.ModuleType("antenv.axon_hooks")
        _h = [None]
        mod.set_axon_ntff_profile_hook = lambda v: _h.__setitem__(0, v)
        mod.get_axon_ntff_profile_hook = lambda: _h[0]
        sys.modules["antenv.axon_hooks"] = mod
        antenv.axon_hooks = mod
    import concourse.bass_utils as bu
    bu.upload_artifacts = lambda d: d
    from trn_agent_boot.trn_boot import _ntff_profile_via_ctypes
    sys.modules["antenv.axon_hooks"].set_axon_ntff_profile_hook(
        _ntff_profile_via_ctypes("/opt/axon/libaxon_pjrt.so"))
